# revision 43
# baseline (speedup 1.0000x reference)
"""Multi-head causal attention (B=4, T=2048, D=1024, H=16) on 8 TRN2 cores.

Tensor-parallel over heads: core c computes heads {2c, 2c+1}; host sums the
8 per-core projection partials (row-split w_proj). Per core:
  - Q', K' feature-major via 512-wide PE matmuls; V token-major directly
    (lhsT = x^T token slice) into persistent vaug tiles whose softmax-ones
    columns are memset once at startup.
  - Score slot = one key block x both heads: two row-tiled PE matmuls
    (h0 rows 0-63, h1 rows 64-127) into the two banks of one PSUM pair
    tile -> they execute concurrently; one wide exp per non-diagonal slot.
  - Causal mask multiply only on the 128-column triangular boundary of
    diagonal blocks (the rest of a diagonal block is fully live).
  - PV uses the ones-augmented V (fused denominator row); normalization via
    reciprocal_approx_fast + gpsimd partition_broadcast + fused DVE mult.
  - Proj drains as bf16 (DVE; chunk-3 on ACT where it overlaps the next
    section's QKV); output DMA'd bf16, summed in f32 on host.
  - Emission is software-pipelined in per-batch sections: QKV(b) and the
    deferred proj(b-2, qc3) interleave as PE filler through attention of
    batch b-1 (scores run 1-1.5 chunks ahead of PV; proj trails norm).
"""

import sys

for _p in ("/opt/trn_rl_repo",):
    if _p not in sys.path:
        sys.path.append(_p)

import numpy as np
import ml_dtypes

B, T, D = 4, 2048, 1024
H = 16
HD = D // H
NORM = float(np.sqrt(D))
N_CORES = 8
HEADS_PER_CORE = H // N_CORES          # 2
FPC = HEADS_PER_CORE * HD              # 128 features per core
QC = 512                               # query chunk
NQC = T // QC                          # 4
KB = 128                               # key block
DKC = D // 128                         # 8 contraction chunks over D
NTB = T // 128                         # 16 token blocks

_BF16 = ml_dtypes.bfloat16

_cache = {}

N_WARM = 85


def _build():
    import concourse.bacc as bacc
    import concourse.mybir as mybir
    from concourse.tile import TileContext
    from concourse.alu_op_type import AluOpType

    f32 = mybir.dt.float32
    bf16 = mybir.dt.bfloat16
    EXP = mybir.ActivationFunctionType.Exp

    nc = bacc.Bacc("TRN2", target_bir_lowering=False, debug=False,
                   num_devices=N_CORES)

    xt = nc.dram_tensor("xt", [B, D, T], bf16, kind="ExternalInput").ap()
    w3 = nc.dram_tensor("w3", [D, 3 * FPC], bf16, kind="ExternalInput").ap()
    wp = nc.dram_tensor("wp", [FPC, D], bf16, kind="ExternalInput").ap()
    masks = nc.dram_tensor("masks", [4, KB, QC], bf16, kind="ExternalInput").ap()
    out = nc.dram_tensor("out", [B, D, T], bf16, kind="ExternalOutput").ap()

    with TileContext(nc) as tc:
        with (
            tc.tile_pool(name="const", bufs=1) as cpool,
            tc.tile_pool(name="xp", bufs=12) as xpool,
            tc.tile_pool(name="qk", bufs=2) as qkpool,
            tc.tile_pool(name="pt", bufs=32) as ptpool,
            tc.tile_pool(name="y", bufs=6) as ypool,
            tc.tile_pool(name="sm", bufs=2) as smpool,
            tc.tile_pool(name="bcp", bufs=2) as bcpool,
            tc.tile_pool(name="ot", bufs=4) as otpool,
            tc.tile_pool(name="psA", bufs=2, space="PSUM") as psA,
            tc.tile_pool(name="psY", bufs=2, space="PSUM") as psY,
            tc.tile_pool(name="psO", bufs=2, space="PSUM") as psO,
        ):
            # ---- constants ----
            w3_t = []
            for kc in range(DKC):
                t = cpool.tile([128, 3 * FPC], bf16, tag=f"w3{kc}")
                nc.sync.dma_start(t[:], w3[kc * 128:(kc + 1) * 128, :])
                w3_t.append(t)
            wp_t = cpool.tile([FPC, D], bf16, tag="wp")
            nc.sync.dma_start(wp_t[:], wp[:])
            mask_t = []
            for p in range(4):
                t = cpool.tile([KB, QC], bf16, tag=f"mask{p}")
                nc.sync.dma_start(t[:], masks[p])
                mask_t.append(t)

            # persistent vaug tiles: [v_h0 64 | ones | v_h1 64 | ones],
            # double-buffered across batches; ones written once here.
            vaug = [[], []]
            for g in range(2):
                for tb in range(NTB):
                    va = cpool.tile([128, 2 * HD + 2], bf16, tag=f"va{g}_{tb}")
                    nc.gpsimd.memset(va[:, HD:HD + 1], 1.0)
                    nc.gpsimd.memset(va[:, 2 * HD + 1:2 * HD + 2], 1.0)
                    vaug[g].append(va)

            # PE warmup on memset tiles (no DMA dependency): keeps the HAM
            # clock-gate busy while the initial x DMA lands.
            wt = cpool.tile([128, QC], bf16, tag="warm")
            nc.vector.memset(wt[:], 0.25)
            psw = psO.tile([128, QC], f32, tag="pso")
            for _ in range(N_WARM):
                nc.tensor.matmul(psw[:], lhsT=wt[:, 0:128], rhs=wt[:],
                                 start=True, stop=True)

            # ---- mutable cross-section state ----
            xp_t = {}      # b -> [8 tiles]
            qkp = {}       # b -> (qp, kp)
            pts = {}       # (a, qc, h, kb) -> (pt tile, j0)
            y_tiles = {}   # (a, qc) -> y tile

            def prefetch_x(b):
                ts = []
                for kc in range(DKC):
                    t = xpool.tile([128, T], bf16, tag="xp")
                    nc.sync.dma_start(t[:], xt[b, kc * 128:(kc + 1) * 128, :])
                    ts.append(t)
                xp_t[b] = ts

            # ---- QKV units ----
            # Note: matmuls of different PSUM accumulation groups must NOT
            # interleave while a group is open (observed data corruption on
            # hardware) — each unit below closes its groups before returning.
            def emit_qk_part(b, c, part):
                if b not in qkp:
                    qp = qkpool.tile([128, T], bf16, tag="qp")
                    kp = qkpool.tile([128, T], bf16, tag="kp")
                    qkp[b] = (qp, kp)
                dst = qkp[b][part]
                ps = psO.tile([128, QC], f32, tag="pso", name="psqk")
                for kc in range(DKC):
                    nc.tensor.matmul(
                        ps[:],
                        lhsT=w3_t[kc][:, 128 * part:128 * (part + 1)],
                        rhs=xp_t[b][kc][:, QC * c:QC * (c + 1)],
                        start=(kc == 0), stop=(kc == DKC - 1),
                    )
                nc.vector.tensor_copy(dst[:, QC * c:QC * (c + 1)], ps[:])

            def emit_v_part(b, c, part):
                # token-major V for two 128-token blocks (lhsT = x^T slice)
                psv = psO.tile([128, QC], f32, tag="pso", name="psv")
                tbs = (4 * c + 2 * part, 4 * c + 2 * part + 1)
                for j, tb in enumerate(tbs):
                    for kc in range(DKC):
                        nc.tensor.matmul(
                            psv[:, 256 * j:256 * j + 128],
                            lhsT=xp_t[b][kc][:, 128 * tb:128 * (tb + 1)],
                            rhs=w3_t[kc][:, 256:384],
                            start=(kc == 0), stop=(kc == DKC - 1),
                        )
                for j, tb in enumerate(tbs):
                    va = vaug[b % 2][tb]
                    nc.vector.tensor_copy(va[:, 0:HD],
                                          psv[:, 256 * j:256 * j + HD])
                    nc.vector.tensor_copy(va[:, HD + 1:2 * HD + 1],
                                          psv[:, 256 * j + HD:256 * j + 128])

            # ---- attention units ----
            def emit_slot(a, qc, kb):
                # one key block, both heads: two concurrent row-tiled MMs
                # into the two banks of one PSUM pair tile, exp on ACT,
                # triangular-boundary mask on DVE.
                j0 = max(0, KB * (kb - 4 * qc))
                p = kb - 4 * qc
                qp, kp = qkp[a][0], qkp[a][1]
                ps = psA.tile([128, 2 * QC], f32, tag="ps")
                for h in range(2):
                    nc.tensor.matmul(
                        ps[:, QC * h + j0:QC * (h + 1)],
                        lhsT=kp[HD * h:HD * (h + 1), KB * kb:KB * (kb + 1)],
                        rhs=qp[HD * h:HD * (h + 1), QC * qc + j0:QC * (qc + 1)],
                        start=True, stop=True,
                    )
                pt = ptpool.tile([KB, 2 * QC], bf16, tag="pt")
                if p < 0:
                    nc.scalar.activation(pt[:], ps[:], EXP, scale=1.0 / NORM)
                else:
                    for h in range(2):
                        nc.scalar.activation(
                            pt[:, QC * h + j0:QC * (h + 1)],
                            ps[:, QC * h + j0:QC * (h + 1)],
                            EXP, scale=1.0 / NORM)
                    for h in range(2):
                        # only the leading 128 columns of a diagonal block
                        # straddle the causal boundary; the rest is unmasked
                        nc.vector.tensor_tensor(
                            pt[:, QC * h + j0:QC * h + j0 + KB],
                            pt[:, QC * h + j0:QC * h + j0 + KB],
                            mask_t[0][:, 0:KB],
                            op=AluOpType.mult,
                        )
                for h in range(2):
                    pts[a, qc, h, kb] = (pt, QC * h, j0)

            def emit_pv(a, qc, h, kbs, psy, nkb, state={}):
                for kb in kbs:
                    pt, off, j0 = pts.pop((a, qc, h, kb))
                    i = state.get((a, qc, h), 0)
                    nc.tensor.matmul(
                        psy[0:HD + 1, j0:QC],
                        lhsT=vaug[a % 2][kb][:, (HD + 1) * h:(HD + 1) * (h + 1)],
                        rhs=pt[:, off + j0:off + QC],
                        start=(i == 0), stop=(i == nkb - 1),
                    )
                    state[a, qc, h] = i + 1

            def emit_drain(a, qc, h, psy, late):
                if (a, qc) not in y_tiles:
                    y_tiles[a, qc] = ypool.tile([FPC, QC], bf16, tag="y", name="y")
                y = y_tiles[a, qc]
                srow = smpool.tile([1, QC], f32, tag=f"srow{h}")
                yu = smpool.tile([HD, QC], f32, tag=f"yu{h}")
                if late:
                    # last-chunk drains run when ACT has no exp work left
                    nc.scalar.copy(srow[:], psy[HD:HD + 1, :])
                    nc.scalar.copy(yu[:], psy[0:HD, :])
                else:
                    nc.vector.tensor_copy(srow[:], psy[HD:HD + 1, :])
                    nc.vector.tensor_copy(yu[:], psy[0:HD, :])
                rec = smpool.tile([1, QC], f32, tag=f"rec{h}")
                nc.vector.reciprocal_approx_fast(rec[:], srow[:])
                bc = bcpool.tile([HD, QC], f32, tag=f"bc{h}")
                nc.gpsimd.partition_broadcast(bc[:], rec[:])
                nc.vector.tensor_tensor(y[HD * h:HD * (h + 1), :], yu[:],
                                        bc[:], op=AluOpType.mult)

            def emit_proj(a, qc, mts, late=False):
                y = y_tiles[a, qc]
                for mt in mts:
                    pso = psO.tile([128, QC], f32, tag="pso")
                    nc.tensor.matmul(
                        pso[:],
                        lhsT=wp_t[:, 128 * mt:128 * (mt + 1)],
                        rhs=y[:],
                        start=True, stop=True,
                    )
                    ot = otpool.tile([128, QC], bf16, tag="ot")
                    if late:
                        # last-chunk proj drains overlap the next section's
                        # QKV phase, where ACT is otherwise idle
                        nc.scalar.copy(ot[:], pso[:])
                    else:
                        nc.vector.tensor_copy(ot[:], pso[:])
                    nc.sync.dma_start(
                        out[a, 128 * mt:128 * (mt + 1), QC * qc:QC * (qc + 1)],
                        ot[:],
                    )

            def kb_order(qc):
                nkb = 4 * (qc + 1)
                return ([kb for kb in range(nkb) if kb < 4 * qc] +
                        [kb for kb in range(nkb) if kb >= 4 * qc])

            def attn_units(a, order=(0, 1, 2, 3)):
                # ordered attention stream for batch a as a list of
                # callables; `order` is the chunk processing sequence (the
                # last entry's drains/proj route to ACT and its proj is
                # deferred/emitted last).
                late_qc = order[3]
                psy_tiles = {}

                def slot_t(qc, kb):
                    return lambda: emit_slot(a, qc, kb)

                def pv_t(qc, h, kbs):
                    kbs = list(kbs)

                    def f():
                        if (qc, h) not in psy_tiles:
                            psy_tiles[qc, h] = psY.tile(
                                [HD + 1, QC], f32, tag="psy", name="psy")
                        emit_pv(a, qc, h, kbs, psy_tiles[qc, h], 4 * (qc + 1))
                    return f

                def drain_t(qc, h):
                    return lambda: emit_drain(a, qc, h, psy_tiles[qc, h],
                                              qc == late_qc)

                def proj_t(qc, mts):
                    mts = list(mts)
                    return lambda: emit_proj(a, qc, mts, qc == late_qc)

                def slots_list(qc):
                    return [slot_t(qc, kb) for kb in range(4 * (qc + 1))]

                def pv_phase(qc, proj_qc):
                    ko = kb_order(qc)
                    groups = [ko[i:i + 3] for i in range(0, len(ko), 3)]
                    out = []
                    for i, grp in enumerate(groups):
                        for h in (0, 1):
                            out.append(pv_t(qc, h, grp))
                            if i == len(groups) - 1:
                                out.append(drain_t(qc, h))
                    if proj_qc is not None:
                        for m in range(0, 8, 2):
                            out.append(proj_t(proj_qc, [m, m + 1]))
                    return out

                def weave(prim, slots):
                    # spread prim units evenly between pacing slots
                    if not prim:
                        return slots
                    if not slots:
                        return prim
                    out = []
                    ratio = len(prim) / len(slots)
                    acc = 0.0
                    pi = 0
                    for s in slots:
                        out.append(s)
                        acc += ratio
                        take = int(acc)
                        out += prim[pi:pi + take]
                        pi += take
                        acc -= take
                    out += prim[pi:]
                    return out

                c0, c1, c2, c3 = order
                units = slots_list(c0) + slots_list(c1)
                units += weave(pv_phase(c0, None), slots_list(c2))
                units += weave(pv_phase(c1, c0), slots_list(c3))
                units += pv_phase(c2, c1)
                units += pv_phase(c3, c2)
                # proj(late_qc) deferred to the next section / tail
                return units

            def qkv_units(b):
                units = []
                for c in range(NQC):
                    units.append(lambda c=c: emit_qk_part(b, c, 0))
                    units.append(lambda c=c: emit_v_part(b, c, 0))
                    units.append(lambda c=c: emit_qk_part(b, c, 1))
                    units.append(lambda c=c: emit_v_part(b, c, 1))
                return units

            def merge(fill, attn):
                # interleave: lead with 2 fillers, then spread the rest
                # evenly through the attention stream.
                seq = []
                lead = fill[:2]
                rest = fill[2:]
                seq += lead
                if not attn:
                    return seq + rest
                if not rest:
                    return seq + attn
                stride = max(1, len(attn) // len(rest))
                ai = 0
                for i, f in enumerate(rest):
                    nxt = min(len(attn), (i + 1) * stride)
                    seq += attn[ai:nxt]
                    seq.append(f)
                    ai = nxt
                seq += attn[ai:]
                return seq

            # ---- sections ----
            prefetch_x(0)
            prefetch_x(1)
            for b in range(B + 1):
                fill = []
                if b < B:
                    fill += qkv_units(b)
                if b >= 2:
                    a2 = b - 2
                    fill.append(lambda a2=a2: emit_proj(a2, 3, [0, 1, 2, 3], True))
                    fill.append(lambda a2=a2: emit_proj(a2, 3, [4, 5, 6, 7], True))
                attn = attn_units(b - 1) if b >= 1 else []
                for u in merge(fill, attn):
                    u()
                if b + 2 <= B - 1:
                    prefetch_x(b + 2)
            # tail: the last batch's deferred chunk-3 proj
            emit_proj(B - 1, 3, list(range(8)), True)

    nc.compile()
    return nc


def _get_nc():
    if "nc" not in _cache:
        _cache["nc"] = _build()
    return _cache["nc"]


def _make_masks():
    i = np.arange(KB)[:, None]
    j = np.arange(QC)[None, :]
    m = np.zeros((4, KB, QC), dtype=np.float32)
    for p in range(4):
        m[p] = (j >= (KB * p + i)).astype(np.float32)
    return m.astype(_BF16)


def shard_inputs(x, w_qkv, w_proj):
    xt = np.ascontiguousarray(np.asarray(x, dtype=np.float32).transpose(0, 2, 1))
    xt = xt.astype(_BF16)
    w_qkv = np.asarray(w_qkv, dtype=np.float32)
    w_proj = np.asarray(w_proj, dtype=np.float32)
    masks = _make_masks()
    in_maps = []
    for c in range(N_CORES):
        qcols = slice(FPC * c, FPC * (c + 1))
        kcols = slice(D + FPC * c, D + FPC * (c + 1))
        vcols = slice(2 * D + FPC * c, 2 * D + FPC * (c + 1))
        w3_c = np.concatenate(
            [w_qkv[:, qcols], w_qkv[:, kcols], w_qkv[:, vcols]], axis=1)
        in_maps.append({
            "xt": xt,
            "w3": np.ascontiguousarray(w3_c).astype(_BF16),
            "wp": np.ascontiguousarray(w_proj[FPC * c:FPC * (c + 1), :]).astype(_BF16),
            "masks": masks,
        })
    return in_maps


def unshard(results):
    total = results[0]["out"].astype(np.float32)
    for r in results[1:]:
        total += r["out"].astype(np.float32)
    return np.ascontiguousarray(total.transpose(0, 2, 1))


def run(inputs, trace=False, **kw):
    from concourse.bass_utils import run_bass_kernel_spmd

    nc = _get_nc()
    in_maps = shard_inputs(inputs["x"], inputs["w_qkv"], inputs["w_proj"])
    res = run_bass_kernel_spmd(nc, in_maps, core_ids=list(range(N_CORES)),
                               trace=trace, **kw)
    return unshard(res.results), res


def kernel(**inputs):
    out, _ = run(inputs, trace=False)
    return out
